# revision 11
# baseline (speedup 1.0000x reference)
"""Trainium2 Bass kernel for nn_ACS (4-branch conv block + top-k channel select).

Strategy:
- Host: top-k of c_score; gather ONLY the 512 surviving output channels;
  fold fuse_weight, all BN affines and the sigmoid scale into conv
  weights / per-channel eviction affines. Output channel permutation is
  applied host-side after gather (free).
- Device (SPMD, 8 cores, 2 images each; no collectives):
  * activations live in a zero-padded [C, 58*58]-style layout; 3x3 convs
    run as 9 shifted matmuls accumulating in PSUM (fp32r: full-rate
    fp32-rounded matmul dtype).
  * branch0 = 3x3 conv; branch1 = 1x1 conv; branch2 = 1x1 conv ->
    BN-affine eviction -> 3x3 conv (+bias evict); branch3 = 1x1 conv ->
    BN-affine eviction -> separable 3x3 sum-pool on VectorE -> affine evict.
  * branch0's ragged tail tile is topped up with branch1 channels
    (center-tap only) so the 128-wide PE columns stay productive.
  * per-image work is split in two 28-row slabs, chunked into 7/8-row
    matmul groups (free dim 406/464 <= one PSUM bank).
"""

import numpy as np

import concourse.mybir as mybir
import concourse.tile as tile
from concourse import bacc
from concourse.bass_utils import run_bass_kernel_spmd

# ---- problem constants (hardcoded per spec) ----
B, C, H, W = 16, 512, 56, 56
MID = 256
NCORES = 8
BL = B // NCORES  # images per core
WP = H + 2  # padded row width 58
SLAB_ROWS = 28
BUF = 30 * WP + 2  # slab buffer free size (30 padded rows + 2 slack) = 1742
EPS = 1e-5
CENTER_FIRST = [4, 0, 1, 2, 3, 5, 6, 7, 8]  # tap order: center tap starts PSUM

F32 = mybir.dt.float32
F32R = mybir.dt.float32r
AF = mybir.ActivationFunctionType

PROFILE = False
LAST_EXEC_NS = None
_CACHE = {}


def _tiles_of(M):
    out = []
    o = 0
    while o < M:
        out.append((o, min(128, M - o)))
        o += 128
    return out


def _mix_split(M0, M1):
    """How many b1 channels ride in b0's ragged tail tile."""
    rem9 = M0 % 128
    t1 = min(128 - rem9, M1) if rem9 else 0
    return rem9, t1


def _par_cols(counts):
    M0, M1, M2, M3 = counts
    n2, n3 = len(_tiles_of(M2)), len(_tiles_of(M3))
    cols = {"s31": 0, "t31": 2}
    c = 4
    cols["a_s1"] = c
    c += n3
    cols["a_t1"] = c
    c += n3
    cols["bias2"] = c
    c += n2
    cols["pscale"] = c
    c += n3
    cols["pbias"] = c
    c += n3
    return cols, c


# ---------------- host-side folding ----------------

def _bn_fold(p):
    g, b, m, v = [p[i].astype(np.float64) for i in range(4)]
    s = g / np.sqrt(v + EPS)
    t = b - m * s
    return s.astype(np.float32), t.astype(np.float32)


def _prep(w_main, w_1x1, w31, bn31, w33, bn33, wa1, bna1, bna2, fuse_weight, c_score):
    f = [float(fuse_weight[i]) for i in range(4)]
    s31, t31 = _bn_fold(bn31)
    s33, t33 = _bn_fold(bn33)
    sa1, ta1 = _bn_fold(bna1)
    sa2, ta2 = _bn_fold(bna2)

    ind = np.argsort(-c_score, kind="stable")[:C]
    scale = (1.0 / (1.0 + np.exp(-c_score.astype(np.float64))))[ind].astype(np.float32)

    js = {b_: [] for b_ in range(4)}
    cs = {b_: [] for b_ in range(4)}
    for j, gi in enumerate(ind):
        b_ = int(gi) // 256
        js[b_].append(j)
        cs[b_].append(int(gi) % 256)
    c0, c1, c2, c3 = [np.array(cs[i], np.int64) for i in range(4)]
    sc = [scale[np.array(js[i], np.int64)] if js[i] else np.zeros(0, np.float32) for i in range(4)]

    W0 = w_main[c0] * f[0] * sc[0][:, None, None, None]            # [M0,C,3,3]
    W1 = w_1x1[c1, :, 0, 0] * f[1] * sc[1][:, None]                # [M1,C]
    W31 = w31[:, :, 0, 0] * f[2]                                   # [MID,C]
    W33 = w33[c2] * (s33[c2] * sc[2])[:, None, None, None]         # [M2,MID,3,3]
    bias2 = t33[c2] * sc[2]
    Wa1 = wa1[c3, :, 0, 0] * f[3]                                  # [M3,C]
    a_s1, a_t1 = sa1[c3], ta1[c3]
    pscale = sa2[c3] * sc[3] / 9.0
    pbias = ta2[c3] * sc[3]

    jperm = np.array(js[0] + js[1] + js[2] + js[3], dtype=np.int64)
    counts = (len(c0), len(c1), len(c2), len(c3))
    M0, M1, M2, M3 = counts

    # branch0 tiles + tail topped up with the first t1 branch1 channels
    rem9, t1 = _mix_split(M0, M1)
    MX = M0 + t1
    WX = np.zeros((MX, C, 3, 3), np.float32)
    WX[:M0] = W0
    if t1:
        WX[M0:, :, 1, 1] = W1[:t1]
    W1rest = W1[t1:]

    def pack_kpm(Wmat, ktiles):
        M = Wmat.shape[0]
        return np.ascontiguousarray(
            Wmat.T.reshape(ktiles, 128, M).transpose(1, 0, 2).reshape(128, ktiles * M)
        ).astype(np.float32)

    def pack_ktpm(Wconv, ktiles):
        M = Wconv.shape[0]
        return np.ascontiguousarray(
            Wconv.transpose(1, 2, 3, 0)
            .reshape(ktiles, 128, 9, M)
            .transpose(1, 0, 2, 3)
            .reshape(128, ktiles * 9 * M)
        ).astype(np.float32)

    arrs = {"W31d": pack_kpm(W31, 4)}
    if MX:
        arrs["WXd"] = pack_ktpm(WX, 4)
    if len(W1rest):
        arrs["W1d"] = pack_kpm(W1rest, 4)
    if M2:
        arrs["W33d"] = pack_ktpm(W33, 2)
    if M3:
        arrs["Wa1d"] = pack_kpm(Wa1, 4)

    cols, ncol = _par_cols(counts)
    par = np.zeros((128, ncol), np.float32)
    par[:, cols["s31"]], par[:, cols["s31"] + 1] = s31[:128], s31[128:]
    par[:, cols["t31"]], par[:, cols["t31"] + 1] = t31[:128], t31[128:]
    for o, (mo, mw) in enumerate(_tiles_of(M3)):
        par[:mw, cols["a_s1"] + o] = a_s1[mo : mo + mw]
        par[:mw, cols["a_t1"] + o] = a_t1[mo : mo + mw]
        par[:mw, cols["pscale"] + o] = pscale[mo : mo + mw]
        par[:mw, cols["pbias"] + o] = pbias[mo : mo + mw]
    for o, (mo, mw) in enumerate(_tiles_of(M2)):
        par[:mw, cols["bias2"] + o] = bias2[mo : mo + mw]
    arrs["par"] = par
    return arrs, counts, jperm


# ---------------- device graph ----------------

def _build(counts):
    M0, M1, M2, M3 = counts
    cols, ncol = _par_cols(counts)
    rem9, t1 = _mix_split(M0, M1)
    MX = M0 + t1
    M1R = M1 - t1
    n3tiles = _tiles_of(M3)
    nc = bacc.Bacc("TRN2", target_bir_lowering=False, debug=False, num_devices=NCORES)

    x_ext = nc.dram_tensor("x", [BL, C, H, W], F32R, kind="ExternalInput")
    WX_ext = nc.dram_tensor("WXd", [128, 4 * 9 * MX], F32R, kind="ExternalInput") if MX else None
    W1_ext = nc.dram_tensor("W1d", [128, 4 * M1R], F32R, kind="ExternalInput") if M1R else None
    W31_ext = nc.dram_tensor("W31d", [128, 4 * MID], F32R, kind="ExternalInput")
    W33_ext = nc.dram_tensor("W33d", [128, 2 * 9 * M2], F32R, kind="ExternalInput") if M2 else None
    Wa1_ext = nc.dram_tensor("Wa1d", [128, 4 * M3], F32R, kind="ExternalInput") if M3 else None
    par_ext = nc.dram_tensor("par", [128, ncol], F32, kind="ExternalInput")
    out_ext = nc.dram_tensor("out", [BL, C, H, W], F32, kind="ExternalOutput")

    off1, off2, off3 = M0, M0 + M1, M0 + M1 + M2

    with tile.TileContext(nc) as tc:
        with (
            tc.tile_pool(name="wpool", bufs=1) as wpool,
            tc.tile_pool(name="acts", bufs=1) as acts,
            tc.tile_pool(name="stage", bufs=4) as stage,
            tc.tile_pool(name="xstage", bufs=4) as xstage,
            tc.tile_pool(name="slabstage", bufs=2) as slabstage,
            tc.tile_pool(name="ps", bufs=8, space="PSUM") as ps,
        ):
            # ---- persistent weights (order: first consumers first) ----
            part = wpool.tile([128, ncol], F32)
            wt31 = wpool.tile([128, 4, 2, 128], F32R)
            wta1 = wpool.tile([128, 4, M3], F32R, name="wta1") if M3 else None
            wtx = wpool.tile([128, 4, 9, MX], F32R, name="wtx") if MX else None
            wt1 = wpool.tile([128, 4, M1R], F32R, name="wt1") if M1R else None
            wt33 = wpool.tile([128, 2, 9, M2], F32R, name="wt33") if M2 else None
            nc.sync.dma_start(part[:], par_ext.ap())
            nc.sync.dma_start(wt31[:], W31_ext.ap().rearrange("p (k o m) -> p k o m", k=4, o=2))
            if M3:
                nc.gpsimd.dma_start(wta1[:], Wa1_ext.ap().rearrange("p (k m) -> p k m", k=4))
            if MX:
                nc.scalar.dma_start(wtx[:], WX_ext.ap().rearrange("p (k t m) -> p k t m", k=4, t=9))
            if M1R:
                nc.gpsimd.dma_start(wt1[:], W1_ext.ap().rearrange("p (k m) -> p k m", k=4))
            if M2:
                nc.scalar.dma_start(wt33[:], W33_ext.ap().rearrange("p (k t m) -> p k t m", k=2, t=9))

            # ---- persistent activation buffers (2 slab slots each) ----
            xt = [[acts.tile([128, BUF], F32R, name=f"xt{k}{s}", tag=f"x{k}s{s}") for s in range(2)] for k in range(4)]
            z1 = [[acts.tile([128, BUF], F32R, name=f"z1{k}{s}", tag=f"z{k}s{s}") for s in range(2)] for k in range(2)]
            za = [[acts.tile([128, BUF], F32, name=f"za{o}{s}", tag=f"za{o}s{s}") for s in range(2)] for o in range(len(n3tiles))]
            hs = [acts.tile([128, BUF], F32, name="hs0", tag="hs0")] * max(1, len(n3tiles))

            # zero only the pad regions (row pads, col pads, slack), not the
            # whole buffers: three tiny memsets per buffer, split over engines.
            def pad_memsets(t, eng):
                a = t[:].bitcast(mybir.dt.uint32)
                eng.memset(a[:, 0:59], 0)  # slack + row 0
                # col pads: w in {0,57} of every row == flat {58r, 58r+1}
                eng.memset(a[:, 0 : 30 * WP].rearrange("p (r w) -> p r w", w=WP)[:, :, 0:2], 0)
                eng.memset(a[:, 29 * WP + 1 : BUF], 0)  # row 29 + tail slack

            engs = [nc.vector, nc.gpsimd]
            ei = 0
            for group in (xt, z1, za):
                for pair in group:
                    for t_ in pair:
                        pad_memsets(t_, engs[ei % 2])
                        ei += 1

            OUT_STARTS = [1, 8, 15, 22]  # slab-local output row starts (7 rows)

            def evict_to_out(acc, mw, segs, b, g0):
                """PSUM rows [0,mw) -> valid cols -> stage; then one DMA per
                (p_lo, p_hi, ch0) segment (PSUM reads must start at part 0)."""
                st = stage.tile([mw, 7 * 56], F32, name="st", tag="st")
                src = acc[0:mw].rearrange("p (r w) -> p r w", w=WP)[:, :, 1:57]
                dst = st[:].rearrange("p (r w) -> p r w", w=56)
                nc.scalar.activation(dst, src, AF.Copy)
                for (p_lo, p_hi, ch0) in segs:
                    nc.scalar.dma_start(
                        out_ext.ap()[b, ch0 : ch0 + p_hi - p_lo, g0 : g0 + 7, :],
                        st[p_lo:p_hi].rearrange("p (r w) -> p r w", w=56),
                    )

            for b in range(BL):
                for s in range(2):
                    # ---- DMA x slab into padded layout (split in 2 pieces) ----
                    src_r0 = 0 if s == 0 else 27
                    l0 = 1 - s
                    for (pr, pn) in [(0, 15), (15, 14)]:
                        for k in range(4):
                            xs = xstage.tile([128, 15 * 56], F32R, name="xs", tag="xs")
                            nc.sync.dma_start(
                                xs[:, 0 : pn * 56],
                                x_ext.ap()[b, 128 * k : 128 * (k + 1), src_r0 + pr : src_r0 + pr + pn, :],
                            )
                            dst = xt[k][s][:, (l0 + pr) * WP + 2 : (l0 + pr) * WP + 2 + pn * WP].rearrange(
                                "p (r w) -> p r w", w=WP
                            )[:, :, 0:56]
                            nc.gpsimd.tensor_copy(
                                dst, xs[:, 0 : pn * 56].rearrange("p (r w) -> p r w", w=56)
                            )

                    if s == 0:
                        mid_chunks = [(1, 7), (8, 7), (15, 7), (22, 8)]
                    else:
                        mid_chunks = [(0, 7), (7, 7), (14, 7), (21, 8)]

                    # ---- branch2 conv1: x -> z1 (full MID channels) ----
                    for (l0c, nr) in mid_chunks:
                        q0 = l0c * WP + 1
                        N = nr * WP
                        for o in range(2):
                            acc = ps.tile([128, N], F32, name="acc", tag="ps")
                            for k in range(4):
                                nc.tensor.matmul(
                                    acc[:], wt31[:, k, o, :], xt[k][s][:, q0 : q0 + N],
                                    start=(k == 0), stop=(k == 3),
                                )
                            src = acc[:].rearrange("p (r w) -> p r w", w=WP)[:, :, 1:57]
                            dst = z1[o][s][:, l0c * WP + 2 : l0c * WP + 2 + N].rearrange(
                                "p (r w) -> p r w", w=WP
                            )[:, :, 0:56]
                            nc.scalar.activation(
                                dst, src, AF.Identity,
                                bias=part[:, cols["t31"] + o : cols["t31"] + o + 1],
                                scale=part[:, cols["s31"] + o : cols["s31"] + o + 1],
                            )

                    # ---- branch3 conv1: x -> za ----
                    for o, (mo, mw) in enumerate(n3tiles):
                        for (l0c, nr) in mid_chunks:
                            q0 = l0c * WP + 1
                            N = nr * WP
                            acc = ps.tile([mw, N], F32, name="acc", tag="ps")
                            for k in range(4):
                                nc.tensor.matmul(
                                    acc[:], wta1[:, k, mo : mo + mw], xt[k][s][:, q0 : q0 + N],
                                    start=(k == 0), stop=(k == 3),
                                )
                            src = acc[:].rearrange("p (r w) -> p r w", w=WP)[:, :, 1:57]
                            dst = za[o][s][:mw, l0c * WP + 2 : l0c * WP + 2 + N].rearrange(
                                "p (r w) -> p r w", w=WP
                            )[:, :, 0:56]
                            nc.scalar.activation(
                                dst, src, AF.Identity,
                                bias=part[:mw, cols["a_t1"] + o : cols["a_t1"] + o + 1],
                                scale=part[:mw, cols["a_s1"] + o : cols["a_s1"] + o + 1],
                            )

                    # ---- branch3 pooling (VectorE separable 3x3 sum) ----
                    add = mybir.AluOpType.add
                    for o, (mo, mw) in enumerate(n3tiles):
                        zz = za[o][s]
                        hh = hs[o]
                        nc.vector.tensor_tensor(
                            hh[:mw, 1 : BUF - 1], zz[:mw, 0 : BUF - 2], zz[:mw, 2:BUF], add
                        )
                        nc.vector.tensor_tensor(
                            hh[:mw, 1 : BUF - 1], hh[:mw, 1 : BUF - 1], zz[:mw, 1 : BUF - 1], add
                        )

                        def v3(buf, base, mwl=mw):
                            return buf[:mwl, base : base + 28 * WP].rearrange(
                                "p (r w) -> p r w", w=WP
                            )[:, :, 0:56]

                        # vsum over valid cols only, in place over za
                        nc.vector.tensor_tensor(v3(zz, 60), v3(hh, 2), v3(hh, 118), add)
                        nc.vector.tensor_tensor(v3(zz, 60), v3(zz, 60), v3(hh, 60), add)
                        st = slabstage.tile([mw, 28 * 56], F32, name="sst", tag="sst")
                        nc.scalar.activation(
                            st[:].rearrange("p (r w) -> p r w", w=56),
                            v3(zz, 60), AF.Identity,
                            bias=part[:mw, cols["pbias"] + o : cols["pbias"] + o + 1],
                            scale=part[:mw, cols["pscale"] + o : cols["pscale"] + o + 1],
                        )
                        g0 = SLAB_ROWS * s
                        nc.scalar.dma_start(
                            out_ext.ap()[b, off3 + mo : off3 + mo + mw, g0 : g0 + 28, :],
                            st[:].rearrange("p (r w) -> p r w", w=56),
                        )

                    # ---- branch0 (+b1 tail riders) and branch1-rest (1x1) ----
                    for l0c in OUT_STARTS:
                        q0 = l0c * WP + 1
                        N = 7 * WP
                        g0 = l0c - 1 + SLAB_ROWS * s
                        for (mo, mw) in _tiles_of(MX):
                            is_mix = mo + mw > M0  # ragged tail tile with b1 riders
                            nb0 = min(mw, M0 - mo)
                            acc = ps.tile([mw, N], F32, name="acc", tag="ps")
                            first = True
                            for t in CENTER_FIRST:
                                dh, dw = t // 3, t % 3
                                qr = q0 + (dh - 1) * WP + (dw - 1)
                                tm = mw if (t == 4 or not is_mix) else nb0
                                if tm == 0:
                                    continue
                                nc.tensor.matmul(
                                    acc[0:tm], wtx[:, 0, t, mo : mo + tm],
                                    xt[0][s][:, qr : qr + N],
                                    start=first, stop=False,
                                )
                                first = False
                                for k in range(1, 4):
                                    nc.tensor.matmul(
                                        acc[0:tm], wtx[:, k, t, mo : mo + tm],
                                        xt[k][s][:, qr : qr + N],
                                        start=False, stop=(t == 8 and k == 3),
                                    )
                            segs = []
                            if nb0:
                                segs.append((0, nb0, mo))
                            if mw > nb0:
                                segs.append((nb0, mw, off1 + (mo + nb0 - M0)))
                            evict_to_out(acc, mw, segs, b, g0)
                        for (mo, mw) in _tiles_of(M1R):
                            acc = ps.tile([mw, N], F32, name="acc", tag="ps")
                            for k in range(4):
                                nc.tensor.matmul(
                                    acc[:], wt1[:, k, mo : mo + mw], xt[k][s][:, q0 : q0 + N],
                                    start=(k == 0), stop=(k == 3),
                                )
                            evict_to_out(acc, mw, [(0, mw, off1 + t1 + mo)], b, g0)

                    # ---- branch2 conv2 (3x3 on z1) ----
                    for l0c in OUT_STARTS:
                        q0 = l0c * WP + 1
                        N = 7 * WP
                        g0 = l0c - 1 + SLAB_ROWS * s
                        for o, (mo, mw) in enumerate(_tiles_of(M2)):
                            acc = ps.tile([mw, N], F32, name="acc", tag="ps")
                            first = True
                            for t in CENTER_FIRST:
                                dh, dw = t // 3, t % 3
                                qr = q0 + (dh - 1) * WP + (dw - 1)
                                for k in range(2):
                                    nc.tensor.matmul(
                                        acc[:], wt33[:, k, t, mo : mo + mw],
                                        z1[k][s][:, qr : qr + N],
                                        start=first, stop=(t == 8 and k == 1),
                                    )
                                    first = False
                            st = stage.tile([mw, 7 * 56], F32, name="st", tag="st")
                            nc.scalar.activation(
                                st[:].rearrange("p (r w) -> p r w", w=56),
                                acc[:].rearrange("p (r w) -> p r w", w=WP)[:, :, 1:57],
                                AF.Identity,
                                bias=part[:mw, cols["bias2"] + o : cols["bias2"] + o + 1],
                            )
                            nc.scalar.dma_start(
                                out_ext.ap()[b, off2 + mo : off2 + mo + mw, g0 : g0 + 7, :],
                                st[:].rearrange("p (r w) -> p r w", w=56),
                            )

    nc.compile()
    return nc


# ---------------- entry point ----------------

def kernel(x, w_main, w_1x1, w31, bn31, w33, bn33, wa1, bna1, bna2, fuse_weight, c_score):
    global LAST_EXEC_NS
    arrs, counts, jperm = _prep(
        w_main, w_1x1, w31, bn31, w33, bn33, wa1, bna1, bna2, fuse_weight, c_score
    )
    if counts not in _CACHE:
        _CACHE[counts] = _build(counts)
    nc = _CACHE[counts]

    x = np.ascontiguousarray(x, dtype=np.float32)
    in_maps = []
    for i in range(NCORES):
        m = {"x": np.ascontiguousarray(x[BL * i : BL * (i + 1)])}
        m.update(arrs)
        in_maps.append(m)

    res = run_bass_kernel_spmd(nc, in_maps, list(range(NCORES)), trace=PROFILE)
    LAST_EXEC_NS = res.exec_time_ns

    full = np.empty((B, C, H, W), np.float32)
    for i in range(NCORES):
        full[BL * i : BL * (i + 1)] = res.results[i]["out"]
    out = np.empty_like(full)
    out[:, jperm] = full
    return out


# revision 13
# speedup vs baseline: 1.0187x; 1.0187x over previous
"""Trainium2 Bass kernel for nn_ACS (4-branch conv block + top-k channel select).

Strategy:
- Host: top-k of c_score; gather ONLY the 512 surviving output channels;
  fold fuse_weight, all BN affines and the sigmoid scale into conv
  weights / per-channel eviction affines. Output channel permutation is
  applied host-side after gather (free).
- Device (SPMD, 8 cores, 2 images each; no collectives):
  * activations live in a zero-padded [C, 58*58]-style layout; 3x3 convs
    run as 9 shifted matmuls accumulating in PSUM (fp32r: full-rate
    fp32-rounded matmul dtype).
  * branch0 = 3x3 conv; branch1 = 1x1 conv; branch2 = 1x1 conv ->
    BN-affine eviction -> 3x3 conv (+bias evict); branch3 = 1x1 conv ->
    BN-affine eviction -> separable 3x3 sum-pool on VectorE -> affine evict.
  * branch0's ragged tail tile is topped up with branch1 channels
    (center-tap only) so the 128-wide PE columns stay productive.
  * per-image work is split in two 28-row slabs, chunked into 7/8-row
    matmul groups (free dim 406/464 <= one PSUM bank).
"""

import numpy as np

import concourse.mybir as mybir
import concourse.tile as tile
from concourse import bacc
from concourse.bass_utils import run_bass_kernel_spmd

# ---- problem constants (hardcoded per spec) ----
B, C, H, W = 16, 512, 56, 56
MID = 256
NCORES = 8
BL = B // NCORES  # images per core
WP = H + 2  # padded row width 58
SLAB_ROWS = 28
BUF = 30 * WP + 2  # slab buffer free size (30 padded rows + 2 slack) = 1742
EPS = 1e-5
CENTER_FIRST = [4, 0, 1, 2, 3, 5, 6, 7, 8]  # tap order: center tap starts PSUM

F32 = mybir.dt.float32
F32R = mybir.dt.float32r
AF = mybir.ActivationFunctionType

PROFILE = False
LAST_EXEC_NS = None
_CACHE = {}


def _tiles_of(M):
    out = []
    o = 0
    while o < M:
        out.append((o, min(128, M - o)))
        o += 128
    return out


def _mix_split(M0, M1):
    """How many b1 channels ride in b0's ragged tail tile."""
    rem9 = M0 % 128
    t1 = min(128 - rem9, M1) if rem9 else 0
    return rem9, t1


def _par_cols(counts):
    M0, M1, M2, M3 = counts
    n2, n3 = len(_tiles_of(M2)), len(_tiles_of(M3))
    cols = {"s31": 0, "t31": 2}
    c = 4
    cols["a_s1"] = c
    c += n3
    cols["a_t1"] = c
    c += n3
    cols["bias2"] = c
    c += n2
    cols["pscale"] = c
    c += n3
    cols["pbias"] = c
    c += n3
    return cols, c


# ---------------- host-side folding ----------------

def _bn_fold(p):
    g, b, m, v = [p[i].astype(np.float64) for i in range(4)]
    s = g / np.sqrt(v + EPS)
    t = b - m * s
    return s.astype(np.float32), t.astype(np.float32)


def _prep(w_main, w_1x1, w31, bn31, w33, bn33, wa1, bna1, bna2, fuse_weight, c_score):
    f = [float(fuse_weight[i]) for i in range(4)]
    s31, t31 = _bn_fold(bn31)
    s33, t33 = _bn_fold(bn33)
    sa1, ta1 = _bn_fold(bna1)
    sa2, ta2 = _bn_fold(bna2)

    ind = np.argsort(-c_score, kind="stable")[:C]
    scale = (1.0 / (1.0 + np.exp(-c_score.astype(np.float64))))[ind].astype(np.float32)

    js = {b_: [] for b_ in range(4)}
    cs = {b_: [] for b_ in range(4)}
    for j, gi in enumerate(ind):
        b_ = int(gi) // 256
        js[b_].append(j)
        cs[b_].append(int(gi) % 256)
    c0, c1, c2, c3 = [np.array(cs[i], np.int64) for i in range(4)]
    sc = [scale[np.array(js[i], np.int64)] if js[i] else np.zeros(0, np.float32) for i in range(4)]

    W0 = w_main[c0] * f[0] * sc[0][:, None, None, None]            # [M0,C,3,3]
    W1 = w_1x1[c1, :, 0, 0] * f[1] * sc[1][:, None]                # [M1,C]
    W31 = w31[:, :, 0, 0] * f[2]                                   # [MID,C]
    W33 = w33[c2] * (s33[c2] * sc[2])[:, None, None, None]         # [M2,MID,3,3]
    bias2 = t33[c2] * sc[2]
    Wa1 = wa1[c3, :, 0, 0] * f[3]                                  # [M3,C]
    a_s1, a_t1 = sa1[c3], ta1[c3]
    pscale = sa2[c3] * sc[3] / 9.0
    pbias = ta2[c3] * sc[3]

    jperm = np.array(js[0] + js[1] + js[2] + js[3], dtype=np.int64)
    counts = (len(c0), len(c1), len(c2), len(c3))
    M0, M1, M2, M3 = counts

    # branch0 tiles + tail topped up with the first t1 branch1 channels
    rem9, t1 = _mix_split(M0, M1)
    MX = M0 + t1
    WX = np.zeros((MX, C, 3, 3), np.float32)
    WX[:M0] = W0
    if t1:
        WX[M0:, :, 1, 1] = W1[:t1]
    W1rest = W1[t1:]

    def pack_kpm(Wmat, ktiles):
        M = Wmat.shape[0]
        return np.ascontiguousarray(
            Wmat.T.reshape(ktiles, 128, M).transpose(1, 0, 2).reshape(128, ktiles * M)
        ).astype(np.float32)

    def pack_ktpm(Wconv, ktiles):
        M = Wconv.shape[0]
        return np.ascontiguousarray(
            Wconv.transpose(1, 2, 3, 0)
            .reshape(ktiles, 128, 9, M)
            .transpose(1, 0, 2, 3)
            .reshape(128, ktiles * 9 * M)
        ).astype(np.float32)

    arrs = {"W31d": pack_kpm(W31, 4)}
    if MX:
        arrs["WXd"] = pack_ktpm(WX, 4)
    if len(W1rest):
        arrs["W1d"] = pack_kpm(W1rest, 4)
    if M2:
        arrs["W33d"] = pack_ktpm(W33, 2)
    if M3:
        arrs["Wa1d"] = pack_kpm(Wa1, 4)

    cols, ncol = _par_cols(counts)
    par = np.zeros((128, ncol), np.float32)
    par[:, cols["s31"]], par[:, cols["s31"] + 1] = s31[:128], s31[128:]
    par[:, cols["t31"]], par[:, cols["t31"] + 1] = t31[:128], t31[128:]
    for o, (mo, mw) in enumerate(_tiles_of(M3)):
        par[:mw, cols["a_s1"] + o] = a_s1[mo : mo + mw]
        par[:mw, cols["a_t1"] + o] = a_t1[mo : mo + mw]
        par[:mw, cols["pscale"] + o] = pscale[mo : mo + mw]
        par[:mw, cols["pbias"] + o] = pbias[mo : mo + mw]
    for o, (mo, mw) in enumerate(_tiles_of(M2)):
        par[:mw, cols["bias2"] + o] = bias2[mo : mo + mw]
    arrs["par"] = par
    return arrs, counts, jperm


# ---------------- device graph ----------------

def _build(counts):
    M0, M1, M2, M3 = counts
    cols, ncol = _par_cols(counts)
    rem9, t1 = _mix_split(M0, M1)
    MX = M0 + t1
    M1R = M1 - t1
    n3tiles = _tiles_of(M3)
    nc = bacc.Bacc("TRN2", target_bir_lowering=False, debug=False, num_devices=NCORES)

    x_ext = nc.dram_tensor("x", [BL, C, H, W], F32R, kind="ExternalInput")
    WX_ext = nc.dram_tensor("WXd", [128, 4 * 9 * MX], F32R, kind="ExternalInput") if MX else None
    W1_ext = nc.dram_tensor("W1d", [128, 4 * M1R], F32R, kind="ExternalInput") if M1R else None
    W31_ext = nc.dram_tensor("W31d", [128, 4 * MID], F32R, kind="ExternalInput")
    W33_ext = nc.dram_tensor("W33d", [128, 2 * 9 * M2], F32R, kind="ExternalInput") if M2 else None
    Wa1_ext = nc.dram_tensor("Wa1d", [128, 4 * M3], F32R, kind="ExternalInput") if M3 else None
    par_ext = nc.dram_tensor("par", [128, ncol], F32, kind="ExternalInput")
    out_ext = nc.dram_tensor("out", [BL, C, H, W], F32, kind="ExternalOutput")

    off1, off2, off3 = M0, M0 + M1, M0 + M1 + M2

    with tile.TileContext(nc) as tc:
        with (
            tc.tile_pool(name="wpool", bufs=1) as wpool,
            tc.tile_pool(name="acts", bufs=1) as acts,
            tc.tile_pool(name="stage", bufs=4) as stage,
            tc.tile_pool(name="xstage", bufs=4) as xstage,
            tc.tile_pool(name="slabstage", bufs=2) as slabstage,
            tc.tile_pool(name="ps", bufs=8, space="PSUM") as ps,
        ):
            # ---- persistent weights (order: first consumers first) ----
            part = wpool.tile([128, ncol], F32)
            wt31 = wpool.tile([128, 4, 2, 128], F32R)
            wta1 = wpool.tile([128, 4, M3], F32R, name="wta1") if M3 else None
            wtx = wpool.tile([128, 4, 9, MX], F32R, name="wtx") if MX else None
            wt1 = wpool.tile([128, 4, M1R], F32R, name="wt1") if M1R else None
            wt33 = wpool.tile([128, 2, 9, M2], F32R, name="wt33") if M2 else None
            nc.sync.dma_start(part[:], par_ext.ap())
            nc.sync.dma_start(wt31[:], W31_ext.ap().rearrange("p (k o m) -> p k o m", k=4, o=2))
            if M3:
                nc.gpsimd.dma_start(wta1[:], Wa1_ext.ap().rearrange("p (k m) -> p k m", k=4))
            if MX:
                nc.scalar.dma_start(wtx[:], WX_ext.ap().rearrange("p (k t m) -> p k t m", k=4, t=9))
            if M1R:
                nc.gpsimd.dma_start(wt1[:], W1_ext.ap().rearrange("p (k m) -> p k m", k=4))
            if M2:
                nc.scalar.dma_start(wt33[:], W33_ext.ap().rearrange("p (k t m) -> p k t m", k=2, t=9))

            # ---- persistent activation buffers (2 slab slots each) ----
            xt = [[acts.tile([128, BUF], F32R, name=f"xt{k}{s}", tag=f"x{k}s{s}") for s in range(2)] for k in range(4)]
            z1 = [[acts.tile([128, BUF], F32R, name=f"z1{k}{s}", tag=f"z{k}s{s}") for s in range(2)] for k in range(2)]
            za = [[acts.tile([128, BUF], F32, name=f"za{o}{s}", tag=f"za{o}s{s}") for s in range(2)] for o in range(len(n3tiles))]
            hs = [acts.tile([128, BUF], F32, name="hs0", tag="hs0")] * max(1, len(n3tiles))

            # zero only the pad regions (row pads, col pads, slack), not the
            # whole buffers: three tiny memsets per buffer, split over engines.
            def pad_memsets(t, eng):
                a = t[:].bitcast(mybir.dt.uint32)
                eng.memset(a[:, 0:59], 0)  # slack + row 0
                # col pads: w in {0,57} of every row == flat {58r, 58r+1}
                eng.memset(a[:, 0 : 30 * WP].rearrange("p (r w) -> p r w", w=WP)[:, :, 0:2], 0)
                eng.memset(a[:, 29 * WP + 1 : BUF], 0)  # row 29 + tail slack

            engs = [nc.vector, nc.gpsimd]
            ei = 0
            for group in (xt, z1, za):
                for pair in group:
                    for t_ in pair:
                        pad_memsets(t_, engs[ei % 2])
                        ei += 1

            OUT_STARTS = [1, 8, 15, 22]  # slab-local output row starts (7 rows)

            def evict_to_out(acc, mw, segs, b, g0):
                """PSUM rows [0,mw) -> valid cols -> stage; then one DMA per
                (p_lo, p_hi, ch0) segment (PSUM reads must start at part 0)."""
                st = stage.tile([mw, 7 * 56], F32, name="st", tag="st")
                src = acc[0:mw].rearrange("p (r w) -> p r w", w=WP)[:, :, 1:57]
                dst = st[:].rearrange("p (r w) -> p r w", w=56)
                nc.scalar.activation(dst, src, AF.Copy)
                for (p_lo, p_hi, ch0) in segs:
                    nc.scalar.dma_start(
                        out_ext.ap()[b, ch0 : ch0 + p_hi - p_lo, g0 : g0 + 7, :],
                        st[p_lo:p_hi].rearrange("p (r w) -> p r w", w=56),
                    )

            for b in range(BL):
                for s in range(2):
                    # ---- DMA x slab into padded layout (split in 2 pieces) ----
                    src_r0 = 0 if s == 0 else 27
                    l0 = 1 - s
                    x_pieces = [(1, 7), (8, 7), (15, 7), (22, 8)] if s == 0 else [(0, 7), (7, 7), (14, 7), (21, 8)]
                    for (lp, pn) in x_pieces:
                        for k in range(4):
                            xs = xstage.tile([128, 8 * 56], F32R, name="xs", tag="xs")
                            nc.sync.dma_start(
                                xs[:, 0 : pn * 56],
                                x_ext.ap()[b, 128 * k : 128 * (k + 1),
                                           SLAB_ROWS * s + lp - 1 : SLAB_ROWS * s + lp - 1 + pn, :],
                            )
                            dst = xt[k][s][:, lp * WP + 2 : lp * WP + 2 + pn * WP].rearrange(
                                "p (r w) -> p r w", w=WP
                            )[:, :, 0:56]
                            srcv = xs[:, 0 : pn * 56].rearrange("p (r w) -> p r w", w=56)
                            if k % 2 == 0:
                                nc.vector.tensor_copy(dst, srcv)
                            else:
                                nc.scalar.activation(dst, srcv, AF.Copy)

                    if s == 0:
                        mid_chunks = [(1, 7), (8, 7), (15, 7), (22, 8)]
                    else:
                        mid_chunks = [(0, 7), (7, 7), (14, 7), (21, 8)]

                    # ---- branch2 conv1: x -> z1 (full MID channels) ----
                    for (l0c, nr) in mid_chunks:
                        q0 = l0c * WP + 1
                        N = nr * WP
                        for o in range(2):
                            acc = ps.tile([128, N], F32, name="acc", tag="ps")
                            for k in range(4):
                                nc.tensor.matmul(
                                    acc[:], wt31[:, k, o, :], xt[k][s][:, q0 : q0 + N],
                                    start=(k == 0), stop=(k == 3),
                                )
                            src = acc[:].rearrange("p (r w) -> p r w", w=WP)[:, :, 1:57]
                            dst = z1[o][s][:, l0c * WP + 2 : l0c * WP + 2 + N].rearrange(
                                "p (r w) -> p r w", w=WP
                            )[:, :, 0:56]
                            nc.scalar.activation(
                                dst, src, AF.Identity,
                                bias=part[:, cols["t31"] + o : cols["t31"] + o + 1],
                                scale=part[:, cols["s31"] + o : cols["s31"] + o + 1],
                            )

                    # ---- branch3 conv1: x -> za ----
                    for o, (mo, mw) in enumerate(n3tiles):
                        for (l0c, nr) in mid_chunks:
                            q0 = l0c * WP + 1
                            N = nr * WP
                            acc = ps.tile([mw, N], F32, name="acc", tag="ps")
                            for k in range(4):
                                nc.tensor.matmul(
                                    acc[:], wta1[:, k, mo : mo + mw], xt[k][s][:, q0 : q0 + N],
                                    start=(k == 0), stop=(k == 3),
                                )
                            src = acc[:].rearrange("p (r w) -> p r w", w=WP)[:, :, 1:57]
                            dst = za[o][s][:mw, l0c * WP + 2 : l0c * WP + 2 + N].rearrange(
                                "p (r w) -> p r w", w=WP
                            )[:, :, 0:56]
                            nc.scalar.activation(
                                dst, src, AF.Identity,
                                bias=part[:mw, cols["a_t1"] + o : cols["a_t1"] + o + 1],
                                scale=part[:mw, cols["a_s1"] + o : cols["a_s1"] + o + 1],
                            )

                    # ---- branch3 pooling (VectorE separable 3x3 sum) ----
                    add = mybir.AluOpType.add
                    for o, (mo, mw) in enumerate(n3tiles):
                        zz = za[o][s]
                        hh = hs[o]
                        nc.vector.tensor_tensor(
                            hh[:mw, 1 : BUF - 1], zz[:mw, 0 : BUF - 2], zz[:mw, 2:BUF], add
                        )
                        nc.vector.tensor_tensor(
                            hh[:mw, 1 : BUF - 1], hh[:mw, 1 : BUF - 1], zz[:mw, 1 : BUF - 1], add
                        )

                        def v3(buf, base, mwl=mw):
                            return buf[:mwl, base : base + 28 * WP].rearrange(
                                "p (r w) -> p r w", w=WP
                            )[:, :, 0:56]

                        # vsum over valid cols only, in place over za
                        nc.vector.tensor_tensor(v3(zz, 60), v3(hh, 2), v3(hh, 118), add)
                        nc.vector.tensor_tensor(v3(zz, 60), v3(zz, 60), v3(hh, 60), add)
                        st = slabstage.tile([mw, 28 * 56], F32, name="sst", tag="sst")
                        nc.scalar.activation(
                            st[:].rearrange("p (r w) -> p r w", w=56),
                            v3(zz, 60), AF.Identity,
                            bias=part[:mw, cols["pbias"] + o : cols["pbias"] + o + 1],
                            scale=part[:mw, cols["pscale"] + o : cols["pscale"] + o + 1],
                        )
                        g0 = SLAB_ROWS * s
                        nc.scalar.dma_start(
                            out_ext.ap()[b, off3 + mo : off3 + mo + mw, g0 : g0 + 28, :],
                            st[:].rearrange("p (r w) -> p r w", w=56),
                        )

                    # ---- branch0 (+b1 tail riders) and branch1-rest (1x1) ----
                    for l0c in OUT_STARTS:
                        q0 = l0c * WP + 1
                        N = 7 * WP
                        g0 = l0c - 1 + SLAB_ROWS * s
                        for (mo, mw) in _tiles_of(MX):
                            is_mix = mo + mw > M0  # ragged tail tile with b1 riders
                            nb0 = min(mw, M0 - mo)
                            acc = ps.tile([mw, N], F32, name="acc", tag="ps")
                            first = True
                            for t in CENTER_FIRST:
                                dh, dw = t // 3, t % 3
                                qr = q0 + (dh - 1) * WP + (dw - 1)
                                tm = mw if (t == 4 or not is_mix) else nb0
                                if tm == 0:
                                    continue
                                nc.tensor.matmul(
                                    acc[0:tm], wtx[:, 0, t, mo : mo + tm],
                                    xt[0][s][:, qr : qr + N],
                                    start=first, stop=False,
                                )
                                first = False
                                for k in range(1, 4):
                                    nc.tensor.matmul(
                                        acc[0:tm], wtx[:, k, t, mo : mo + tm],
                                        xt[k][s][:, qr : qr + N],
                                        start=False, stop=(t == 8 and k == 3),
                                    )
                            segs = []
                            if nb0:
                                segs.append((0, nb0, mo))
                            if mw > nb0:
                                segs.append((nb0, mw, off1 + (mo + nb0 - M0)))
                            evict_to_out(acc, mw, segs, b, g0)
                        for (mo, mw) in _tiles_of(M1R):
                            acc = ps.tile([mw, N], F32, name="acc", tag="ps")
                            for k in range(4):
                                nc.tensor.matmul(
                                    acc[:], wt1[:, k, mo : mo + mw], xt[k][s][:, q0 : q0 + N],
                                    start=(k == 0), stop=(k == 3),
                                )
                            evict_to_out(acc, mw, [(0, mw, off1 + t1 + mo)], b, g0)

                    # ---- branch2 conv2 (3x3 on z1) ----
                    for l0c in OUT_STARTS:
                        q0 = l0c * WP + 1
                        N = 7 * WP
                        g0 = l0c - 1 + SLAB_ROWS * s
                        for o, (mo, mw) in enumerate(_tiles_of(M2)):
                            acc = ps.tile([mw, N], F32, name="acc", tag="ps")
                            first = True
                            for t in CENTER_FIRST:
                                dh, dw = t // 3, t % 3
                                qr = q0 + (dh - 1) * WP + (dw - 1)
                                for k in range(2):
                                    nc.tensor.matmul(
                                        acc[:], wt33[:, k, t, mo : mo + mw],
                                        z1[k][s][:, qr : qr + N],
                                        start=first, stop=(t == 8 and k == 1),
                                    )
                                    first = False
                            st = stage.tile([mw, 7 * 56], F32, name="st", tag="st")
                            nc.scalar.activation(
                                st[:].rearrange("p (r w) -> p r w", w=56),
                                acc[:].rearrange("p (r w) -> p r w", w=WP)[:, :, 1:57],
                                AF.Identity,
                                bias=part[:mw, cols["bias2"] + o : cols["bias2"] + o + 1],
                            )
                            nc.scalar.dma_start(
                                out_ext.ap()[b, off2 + mo : off2 + mo + mw, g0 : g0 + 7, :],
                                st[:].rearrange("p (r w) -> p r w", w=56),
                            )

    nc.compile()
    return nc


# ---------------- entry point ----------------

def kernel(x, w_main, w_1x1, w31, bn31, w33, bn33, wa1, bna1, bna2, fuse_weight, c_score):
    global LAST_EXEC_NS
    arrs, counts, jperm = _prep(
        w_main, w_1x1, w31, bn31, w33, bn33, wa1, bna1, bna2, fuse_weight, c_score
    )
    if counts not in _CACHE:
        _CACHE[counts] = _build(counts)
    nc = _CACHE[counts]

    x = np.ascontiguousarray(x, dtype=np.float32)
    in_maps = []
    for i in range(NCORES):
        m = {"x": np.ascontiguousarray(x[BL * i : BL * (i + 1)])}
        m.update(arrs)
        in_maps.append(m)

    res = run_bass_kernel_spmd(nc, in_maps, list(range(NCORES)), trace=PROFILE)
    LAST_EXEC_NS = res.exec_time_ns

    full = np.empty((B, C, H, W), np.float32)
    for i in range(NCORES):
        full[BL * i : BL * (i + 1)] = res.results[i]["out"]
    out = np.empty_like(full)
    out[:, jperm] = full
    return out


# revision 15
# speedup vs baseline: 1.1833x; 1.1616x over previous
"""Trainium2 Bass kernel for nn_ACS (4-branch conv block + top-k channel select).

Strategy:
- Host: top-k of c_score; gather ONLY the 512 surviving output channels;
  fold fuse_weight, all BN affines and the sigmoid scale into conv
  weights / per-channel eviction affines. Output channel permutation is
  applied host-side after gather (free).
- Device (SPMD, 8 cores, 2 images each; no collectives):
  * activations live in a zero-padded [C, 58*58]-style layout; 3x3 convs
    run as 9 shifted matmuls accumulating in PSUM (fp32r: full-rate
    fp32-rounded matmul dtype).
  * branch0 = 3x3 conv; branch1 = 1x1 conv; branch2 = 1x1 conv ->
    BN-affine eviction -> 3x3 conv (+bias evict); branch3 = 1x1 conv ->
    BN-affine eviction -> separable 3x3 sum-pool on VectorE -> affine evict.
  * branch0's ragged tail tile is topped up with branch1 channels
    (center-tap only) so the 128-wide PE columns stay productive.
  * per-image work is split in two 28-row slabs, chunked into 7/8-row
    matmul groups (free dim 406/464 <= one PSUM bank).
"""

import numpy as np

import concourse.mybir as mybir
import concourse.tile as tile
from concourse import bacc
from concourse.bass_utils import run_bass_kernel_spmd

# ---- problem constants (hardcoded per spec) ----
B, C, H, W = 16, 512, 56, 56
MID = 256
NCORES = 8
BL = B // NCORES  # images per core
WP = H + 2  # padded row width 58
SLAB_ROWS = 28
BUF = 30 * WP + 2  # slab buffer free size (30 padded rows + 2 slack) = 1742
EPS = 1e-5
CENTER_FIRST = [4, 0, 1, 2, 3, 5, 6, 7, 8]  # tap order: center tap starts PSUM

F32 = mybir.dt.float32
F32R = mybir.dt.float32r
AF = mybir.ActivationFunctionType

PROFILE = False
LAST_EXEC_NS = None
_CACHE = {}


def _tiles_of(M):
    out = []
    o = 0
    while o < M:
        out.append((o, min(128, M - o)))
        o += 128
    return out


def _e_split(M0):
    """b0 tail: use tap-expansion when the ragged remainder is small."""
    rem9 = M0 % 128
    use_e = 0 < rem9 <= 28
    return (M0 - rem9, rem9) if use_e else (M0, 0)


def _par_cols(counts):
    M0, M1, M2, M3 = counts
    n2, n3 = len(_tiles_of(M2)), len(_tiles_of(M3))
    cols = {"s31": 0, "t31": 2}
    c = 4
    cols["a_s1"] = c
    c += n3
    cols["a_t1"] = c
    c += n3
    cols["bias2"] = c
    c += n2
    cols["pscale"] = c
    c += n3
    cols["pbias"] = c
    c += n3
    return cols, c


# ---------------- host-side folding ----------------

def _bn_fold(p):
    g, b, m, v = [p[i].astype(np.float64) for i in range(4)]
    s = g / np.sqrt(v + EPS)
    t = b - m * s
    return s.astype(np.float32), t.astype(np.float32)


def _prep(w_main, w_1x1, w31, bn31, w33, bn33, wa1, bna1, bna2, fuse_weight, c_score):
    f = [float(fuse_weight[i]) for i in range(4)]
    s31, t31 = _bn_fold(bn31)
    s33, t33 = _bn_fold(bn33)
    sa1, ta1 = _bn_fold(bna1)
    sa2, ta2 = _bn_fold(bna2)

    ind = np.argsort(-c_score, kind="stable")[:C]
    scale = (1.0 / (1.0 + np.exp(-c_score.astype(np.float64))))[ind].astype(np.float32)

    js = {b_: [] for b_ in range(4)}
    cs = {b_: [] for b_ in range(4)}
    for j, gi in enumerate(ind):
        b_ = int(gi) // 256
        js[b_].append(j)
        cs[b_].append(int(gi) % 256)
    c0, c1, c2, c3 = [np.array(cs[i], np.int64) for i in range(4)]
    sc = [scale[np.array(js[i], np.int64)] if js[i] else np.zeros(0, np.float32) for i in range(4)]

    W0 = w_main[c0] * f[0] * sc[0][:, None, None, None]            # [M0,C,3,3]
    W1 = w_1x1[c1, :, 0, 0] * f[1] * sc[1][:, None]                # [M1,C]
    W31 = w31[:, :, 0, 0] * f[2]                                   # [MID,C]
    W33 = w33[c2] * (s33[c2] * sc[2])[:, None, None, None]         # [M2,MID,3,3]
    bias2 = t33[c2] * sc[2]
    Wa1 = wa1[c3, :, 0, 0] * f[3]                                  # [M3,C]
    a_s1, a_t1 = sa1[c3], ta1[c3]
    pscale = sa2[c3] * sc[3] / 9.0
    pbias = ta2[c3] * sc[3]

    jperm = np.array(js[0] + js[1] + js[2] + js[3], dtype=np.int64)
    counts = (len(c0), len(c1), len(c2), len(c3))
    M0, M1, M2, M3 = counts

    # branch0: full 128-wide tiles; small ragged tail handled via tap
    # expansion (e-channels) instead of a nearly-empty 9-tap tile
    rem9 = M0 % 128
    use_e = 0 < rem9 <= 28
    M0F = M0 - rem9 if use_e else M0
    W0F = W0[:M0F]
    if use_e:
        W0T = W0[M0F:]  # [rem9, C, 3, 3]
        # e-channel 1x1 weights: channel (t*rem9 + j) = tap t of tail ch j
        Ew = np.ascontiguousarray(
            W0T.transpose(2, 3, 0, 1).reshape(9 * rem9, C)
        )
        # selector for the tail conv: lhsT[i, t, j] = 1 iff i == t*rem9+j
        sel = np.zeros((128, 9, rem9), np.float32)
        for t_ in range(9):
            for j_ in range(rem9):
                sel[t_ * rem9 + j_, t_, j_] = 1.0

    def pack_kpm(Wmat, ktiles):
        M = Wmat.shape[0]
        return np.ascontiguousarray(
            Wmat.T.reshape(ktiles, 128, M).transpose(1, 0, 2).reshape(128, ktiles * M)
        ).astype(np.float32)

    def pack_ktpm(Wconv, ktiles):
        M = Wconv.shape[0]
        return np.ascontiguousarray(
            Wconv.transpose(1, 2, 3, 0)
            .reshape(ktiles, 128, 9, M)
            .transpose(1, 0, 2, 3)
            .reshape(128, ktiles * 9 * M)
        ).astype(np.float32)

    arrs = {"W31d": pack_kpm(W31, 4)}
    if M0F:
        arrs["W0d"] = pack_ktpm(W0F, 4)
    if use_e:
        arrs["Ewd"] = pack_kpm(Ew, 4)
        arrs["seld"] = np.ascontiguousarray(sel.reshape(128, 9 * rem9))
    if M1:
        arrs["W1d"] = pack_kpm(W1, 4)
    if M2:
        arrs["W33d"] = pack_ktpm(W33, 2)
    if M3:
        arrs["Wa1d"] = pack_kpm(Wa1, 4)

    cols, ncol = _par_cols(counts)
    par = np.zeros((128, ncol), np.float32)
    par[:, cols["s31"]], par[:, cols["s31"] + 1] = s31[:128], s31[128:]
    par[:, cols["t31"]], par[:, cols["t31"] + 1] = t31[:128], t31[128:]
    for o, (mo, mw) in enumerate(_tiles_of(M3)):
        par[:mw, cols["a_s1"] + o] = a_s1[mo : mo + mw]
        par[:mw, cols["a_t1"] + o] = a_t1[mo : mo + mw]
        par[:mw, cols["pscale"] + o] = pscale[mo : mo + mw]
        par[:mw, cols["pbias"] + o] = pbias[mo : mo + mw]
    for o, (mo, mw) in enumerate(_tiles_of(M2)):
        par[:mw, cols["bias2"] + o] = bias2[mo : mo + mw]
    arrs["par"] = par
    return arrs, counts, jperm


# ---------------- device graph ----------------

def _build(counts):
    M0, M1, M2, M3 = counts
    cols, ncol = _par_cols(counts)
    M0F, rem9 = _e_split(M0)
    NE = 9 * rem9  # e-channel count
    n3tiles = _tiles_of(M3)
    nc = bacc.Bacc("TRN2", target_bir_lowering=False, debug=False, num_devices=NCORES)

    x_ext = nc.dram_tensor("x", [BL, C, H, W], F32R, kind="ExternalInput")
    W0_ext = nc.dram_tensor("W0d", [128, 4 * 9 * M0F], F32R, kind="ExternalInput") if M0F else None
    Ew_ext = nc.dram_tensor("Ewd", [128, 4 * NE], F32R, kind="ExternalInput") if rem9 else None
    sel_ext = nc.dram_tensor("seld", [128, NE], F32R, kind="ExternalInput") if rem9 else None
    W1_ext = nc.dram_tensor("W1d", [128, 4 * M1], F32R, kind="ExternalInput") if M1 else None
    W31_ext = nc.dram_tensor("W31d", [128, 4 * MID], F32R, kind="ExternalInput")
    W33_ext = nc.dram_tensor("W33d", [128, 2 * 9 * M2], F32R, kind="ExternalInput") if M2 else None
    Wa1_ext = nc.dram_tensor("Wa1d", [128, 4 * M3], F32R, kind="ExternalInput") if M3 else None
    par_ext = nc.dram_tensor("par", [128, ncol], F32, kind="ExternalInput")
    out_ext = nc.dram_tensor("out", [BL, C, H, W], F32, kind="ExternalOutput")

    off1, off2, off3 = M0, M0 + M1, M0 + M1 + M2

    with tile.TileContext(nc) as tc:
        with (
            tc.tile_pool(name="wpool", bufs=1) as wpool,
            tc.tile_pool(name="acts", bufs=1) as acts,
            tc.tile_pool(name="stage", bufs=4) as stage,
            tc.tile_pool(name="xstage", bufs=4) as xstage,
            tc.tile_pool(name="slabstage", bufs=1) as slabstage,
            tc.tile_pool(name="ps", bufs=8, space="PSUM") as ps,
        ):
            # ---- persistent weights (order: first consumers first) ----
            part = wpool.tile([128, ncol], F32)
            wt31 = wpool.tile([128, 4, 2, 128], F32R)
            wta1 = wpool.tile([128, 4, M3], F32R, name="wta1") if M3 else None
            wt0 = wpool.tile([128, 4, 9, M0F], F32R, name="wt0") if M0F else None
            wte = wpool.tile([128, 4, NE], F32R, name="wte") if rem9 else None
            selt = wpool.tile([128, 9, rem9], F32R, name="selt") if rem9 else None
            wt1 = wpool.tile([128, 4, M1], F32R, name="wt1") if M1 else None
            wt33 = wpool.tile([128, 2, 9, M2], F32R, name="wt33") if M2 else None
            nc.sync.dma_start(part[:], par_ext.ap())
            nc.sync.dma_start(wt31[:], W31_ext.ap().rearrange("p (k o m) -> p k o m", k=4, o=2))
            if M3:
                nc.gpsimd.dma_start(wta1[:], Wa1_ext.ap().rearrange("p (k m) -> p k m", k=4))
            if M0F:
                nc.scalar.dma_start(wt0[:], W0_ext.ap().rearrange("p (k t m) -> p k t m", k=4, t=9))
            if rem9:
                nc.gpsimd.dma_start(wte[:], Ew_ext.ap().rearrange("p (k m) -> p k m", k=4))
                nc.gpsimd.dma_start(selt[:], sel_ext.ap().rearrange("p (t m) -> p t m", t=9))
            if M1:
                nc.gpsimd.dma_start(wt1[:], W1_ext.ap().rearrange("p (k m) -> p k m", k=4))
            if M2:
                nc.scalar.dma_start(wt33[:], W33_ext.ap().rearrange("p (k t m) -> p k t m", k=2, t=9))

            # ---- persistent activation buffers (2 slab slots each) ----
            xt = [[acts.tile([128, BUF], F32R, name=f"xt{k}{s}", tag=f"x{k}s{s}") for s in range(2)] for k in range(4)]
            z1 = [[acts.tile([128, BUF], F32R, name=f"z1{k}{s}", tag=f"z{k}s{s}") for s in range(2)] for k in range(2)]
            za = [[acts.tile([128, BUF], F32, name=f"za{o}{s}", tag=f"za{o}s{s}") for s in range(2)] for o in range(len(n3tiles))]
            ze = [acts.tile([128, BUF], F32R, name=f"ze{s}", tag=f"zes{s}") for s in range(2)] if rem9 else None
            hs = [acts.tile([128, BUF], F32, name="hs0", tag="hs0")] * max(1, len(n3tiles))

            # zero only the pad regions (row pads, col pads, slack), not the
            # whole buffers: three tiny memsets per buffer, split over engines.
            def pad_memsets(t, eng):
                a = t[:].bitcast(mybir.dt.uint32)
                eng.memset(a[:, 0:59], 0)  # slack + row 0
                # col pads: w in {0,57} of every row == flat {58r, 58r+1}
                eng.memset(a[:, 0 : 30 * WP].rearrange("p (r w) -> p r w", w=WP)[:, :, 0:2], 0)
                eng.memset(a[:, 29 * WP + 1 : BUF], 0)  # row 29 + tail slack

            for group in (xt, z1, za, [ze] if rem9 else []):
                for pair in group:
                    for t_ in pair:
                        pad_memsets(t_, nc.vector)

            OUT_STARTS = [1, 8, 15, 22]  # slab-local output row starts (7 rows)

            def evict_to_out(acc, mw, segs, b, g0):
                """PSUM rows [0,mw) -> valid cols -> stage; then one DMA per
                (p_lo, p_hi, ch0) segment (PSUM reads must start at part 0)."""
                st = stage.tile([mw, 7 * 56], F32, name="st", tag="st")
                src = acc[0:mw].rearrange("p (r w) -> p r w", w=WP)[:, :, 1:57]
                dst = st[:].rearrange("p (r w) -> p r w", w=56)
                nc.scalar.activation(dst, src, AF.Copy)
                for (p_lo, p_hi, ch0) in segs:
                    nc.scalar.dma_start(
                        out_ext.ap()[b, ch0 : ch0 + p_hi - p_lo, g0 : g0 + 7, :],
                        st[p_lo:p_hi].rearrange("p (r w) -> p r w", w=56),
                    )

            for b in range(BL):
                for s in range(2):
                    # ---- DMA x slab into padded layout (split in 2 pieces) ----
                    src_r0 = 0 if s == 0 else 27
                    l0 = 1 - s
                    x_pieces = [(1, 7), (8, 7), (15, 7), (22, 8)] if s == 0 else [(0, 7), (7, 7), (14, 7), (21, 8)]
                    for (lp, pn) in x_pieces:
                        for k in range(4):
                            xs = xstage.tile([128, 8 * 56], F32R, name="xs", tag="xs")
                            nc.sync.dma_start(
                                xs[:, 0 : pn * 56],
                                x_ext.ap()[b, 128 * k : 128 * (k + 1),
                                           SLAB_ROWS * s + lp - 1 : SLAB_ROWS * s + lp - 1 + pn, :],
                            )
                            dst = xt[k][s][:, lp * WP + 2 : lp * WP + 2 + pn * WP].rearrange(
                                "p (r w) -> p r w", w=WP
                            )[:, :, 0:56]
                            srcv = xs[:, 0 : pn * 56].rearrange("p (r w) -> p r w", w=56)
                            if k % 2 == 0:
                                nc.vector.tensor_copy(dst, srcv)
                            else:
                                nc.scalar.activation(dst, srcv, AF.Copy)

                    if s == 0:
                        mid_chunks = [(1, 7), (8, 7), (15, 7), (22, 8)]
                    else:
                        mid_chunks = [(0, 7), (7, 7), (14, 7), (21, 8)]

                    # ---- branch2 conv1: x -> z1 (full MID channels) ----
                    for (l0c, nr) in mid_chunks:
                        q0 = l0c * WP + 1
                        N = nr * WP
                        for o in range(2):
                            acc = ps.tile([128, N], F32, name="acc", tag="ps")
                            for k in range(4):
                                nc.tensor.matmul(
                                    acc[:], wt31[:, k, o, :], xt[k][s][:, q0 : q0 + N],
                                    start=(k == 0), stop=(k == 3),
                                )
                            src = acc[:].rearrange("p (r w) -> p r w", w=WP)[:, :, 1:57]
                            dst = z1[o][s][:, l0c * WP + 2 : l0c * WP + 2 + N].rearrange(
                                "p (r w) -> p r w", w=WP
                            )[:, :, 0:56]
                            nc.scalar.activation(
                                dst, src, AF.Identity,
                                bias=part[:, cols["t31"] + o : cols["t31"] + o + 1],
                                scale=part[:, cols["s31"] + o : cols["s31"] + o + 1],
                            )

                    # ---- branch3 conv1: x -> za ----
                    for o, (mo, mw) in enumerate(n3tiles):
                        for (l0c, nr) in mid_chunks:
                            q0 = l0c * WP + 1
                            N = nr * WP
                            acc = ps.tile([mw, N], F32, name="acc", tag="ps")
                            for k in range(4):
                                nc.tensor.matmul(
                                    acc[:], wta1[:, k, mo : mo + mw], xt[k][s][:, q0 : q0 + N],
                                    start=(k == 0), stop=(k == 3),
                                )
                            src = acc[:].rearrange("p (r w) -> p r w", w=WP)[:, :, 1:57]
                            dst = za[o][s][:mw, l0c * WP + 2 : l0c * WP + 2 + N].rearrange(
                                "p (r w) -> p r w", w=WP
                            )[:, :, 0:56]
                            nc.scalar.activation(
                                dst, src, AF.Identity,
                                bias=part[:mw, cols["a_t1"] + o : cols["a_t1"] + o + 1],
                                scale=part[:mw, cols["a_s1"] + o : cols["a_s1"] + o + 1],
                            )

                    # ---- e-channels for b0's tail (1x1 conv on mid-chunks) ----
                    if rem9:
                        for (l0c, nr) in mid_chunks:
                            q0 = l0c * WP + 1
                            N = nr * WP
                            acc = ps.tile([NE, N], F32, name="acc", tag="ps")
                            for k in range(4):
                                nc.tensor.matmul(
                                    acc[:], wte[:, k, :], xt[k][s][:, q0 : q0 + N],
                                    start=(k == 0), stop=(k == 3),
                                )
                            src = acc[:].rearrange("p (r w) -> p r w", w=WP)[:, :, 1:57]
                            dst = ze[s][:NE, l0c * WP + 2 : l0c * WP + 2 + N].rearrange(
                                "p (r w) -> p r w", w=WP
                            )[:, :, 0:56]
                            nc.scalar.activation(dst, src, AF.Copy)

                    # ---- branch3 pooling (VectorE separable 3x3 sum) ----
                    add = mybir.AluOpType.add
                    for o, (mo, mw) in enumerate(n3tiles):
                        zz = za[o][s]
                        hh = hs[o]
                        nc.vector.tensor_tensor(
                            hh[:mw, 1 : BUF - 1], zz[:mw, 0 : BUF - 2], zz[:mw, 2:BUF], add
                        )
                        nc.vector.tensor_tensor(
                            hh[:mw, 1 : BUF - 1], hh[:mw, 1 : BUF - 1], zz[:mw, 1 : BUF - 1], add
                        )

                        def v3(buf, base, mwl=mw):
                            return buf[:mwl, base : base + 28 * WP].rearrange(
                                "p (r w) -> p r w", w=WP
                            )[:, :, 0:56]

                        # vsum over valid cols only, in place over za
                        nc.vector.tensor_tensor(v3(zz, 60), v3(hh, 2), v3(hh, 118), add)
                        nc.vector.tensor_tensor(v3(zz, 60), v3(zz, 60), v3(hh, 60), add)
                        st = slabstage.tile([mw, 28 * 56], F32, name="sst", tag="sst")
                        nc.scalar.activation(
                            st[:].rearrange("p (r w) -> p r w", w=56),
                            v3(zz, 60), AF.Identity,
                            bias=part[:mw, cols["pbias"] + o : cols["pbias"] + o + 1],
                            scale=part[:mw, cols["pscale"] + o : cols["pscale"] + o + 1],
                        )
                        g0 = SLAB_ROWS * s
                        nc.scalar.dma_start(
                            out_ext.ap()[b, off3 + mo : off3 + mo + mw, g0 : g0 + 28, :],
                            st[:].rearrange("p (r w) -> p r w", w=56),
                        )

                    # ---- branch0 (+b1 tail riders) and branch1-rest (1x1) ----
                    for l0c in OUT_STARTS:
                        q0 = l0c * WP + 1
                        N = 7 * WP
                        g0 = l0c - 1 + SLAB_ROWS * s
                        for (mo, mw) in _tiles_of(M0F):
                            acc = ps.tile([mw, N], F32, name="acc", tag="ps")
                            first = True
                            for t in CENTER_FIRST:
                                dh, dw = t // 3, t % 3
                                qr = q0 + (dh - 1) * WP + (dw - 1)
                                for k in range(4):
                                    nc.tensor.matmul(
                                        acc[:], wt0[:, k, t, mo : mo + mw],
                                        xt[k][s][:, qr : qr + N],
                                        start=first, stop=(t == 8 and k == 3),
                                    )
                                    first = False
                            evict_to_out(acc, mw, [(0, mw, mo)], b, g0)
                        if rem9:
                            # tail channels: 9-tap conv over the e-buffer with
                            # one-hot selector weights (K = NE)
                            acc = ps.tile([rem9, N], F32, name="acc", tag="ps")
                            first = True
                            for t in CENTER_FIRST:
                                dh, dw = t // 3, t % 3
                                qr = q0 + (dh - 1) * WP + (dw - 1)
                                nc.tensor.matmul(
                                    acc[:], selt[:NE, t, :], ze[s][:NE, qr : qr + N],
                                    start=first, stop=(t == 8),
                                )
                                first = False
                            evict_to_out(acc, rem9, [(0, rem9, M0F)], b, g0)
                        for (mo, mw) in _tiles_of(M1):
                            acc = ps.tile([mw, N], F32, name="acc", tag="ps")
                            for k in range(4):
                                nc.tensor.matmul(
                                    acc[:], wt1[:, k, mo : mo + mw], xt[k][s][:, q0 : q0 + N],
                                    start=(k == 0), stop=(k == 3),
                                )
                            evict_to_out(acc, mw, [(0, mw, off1 + mo)], b, g0)

                    # ---- branch2 conv2 (3x3 on z1) ----
                    for l0c in OUT_STARTS:
                        q0 = l0c * WP + 1
                        N = 7 * WP
                        g0 = l0c - 1 + SLAB_ROWS * s
                        for o, (mo, mw) in enumerate(_tiles_of(M2)):
                            acc = ps.tile([mw, N], F32, name="acc", tag="ps")
                            first = True
                            for t in CENTER_FIRST:
                                dh, dw = t // 3, t % 3
                                qr = q0 + (dh - 1) * WP + (dw - 1)
                                for k in range(2):
                                    nc.tensor.matmul(
                                        acc[:], wt33[:, k, t, mo : mo + mw],
                                        z1[k][s][:, qr : qr + N],
                                        start=first, stop=(t == 8 and k == 1),
                                    )
                                    first = False
                            st = stage.tile([mw, 7 * 56], F32, name="st", tag="st")
                            nc.scalar.activation(
                                st[:].rearrange("p (r w) -> p r w", w=56),
                                acc[:].rearrange("p (r w) -> p r w", w=WP)[:, :, 1:57],
                                AF.Identity,
                                bias=part[:mw, cols["bias2"] + o : cols["bias2"] + o + 1],
                            )
                            nc.scalar.dma_start(
                                out_ext.ap()[b, off2 + mo : off2 + mo + mw, g0 : g0 + 7, :],
                                st[:].rearrange("p (r w) -> p r w", w=56),
                            )

    nc.compile()
    return nc


# ---------------- entry point ----------------

def kernel(x, w_main, w_1x1, w31, bn31, w33, bn33, wa1, bna1, bna2, fuse_weight, c_score):
    global LAST_EXEC_NS
    arrs, counts, jperm = _prep(
        w_main, w_1x1, w31, bn31, w33, bn33, wa1, bna1, bna2, fuse_weight, c_score
    )
    if counts not in _CACHE:
        _CACHE[counts] = _build(counts)
    nc = _CACHE[counts]

    x = np.ascontiguousarray(x, dtype=np.float32)
    in_maps = []
    for i in range(NCORES):
        m = {"x": np.ascontiguousarray(x[BL * i : BL * (i + 1)])}
        m.update(arrs)
        in_maps.append(m)

    res = run_bass_kernel_spmd(nc, in_maps, list(range(NCORES)), trace=PROFILE)
    LAST_EXEC_NS = res.exec_time_ns

    full = np.empty((B, C, H, W), np.float32)
    for i in range(NCORES):
        full[BL * i : BL * (i + 1)] = res.results[i]["out"]
    out = np.empty_like(full)
    out[:, jperm] = full
    return out


# revision 16
# speedup vs baseline: 1.1857x; 1.0021x over previous
"""Trainium2 Bass kernel for nn_ACS (4-branch conv block + top-k channel select).

Strategy:
- Host: top-k of c_score; gather ONLY the 512 surviving output channels;
  fold fuse_weight, all BN affines and the sigmoid scale into conv
  weights / per-channel eviction affines. Output channel permutation is
  applied host-side after gather (free).
- Device (SPMD, 8 cores, 2 images each; no collectives):
  * activations live in a zero-padded [C, 58*58]-style layout; 3x3 convs
    run as 9 shifted matmuls accumulating in PSUM (fp32r: full-rate
    fp32-rounded matmul dtype).
  * branch0 = 3x3 conv; branch1 = 1x1 conv; branch2 = 1x1 conv ->
    BN-affine eviction -> 3x3 conv (+bias evict); branch3 = 1x1 conv ->
    BN-affine eviction -> separable 3x3 sum-pool on VectorE -> affine evict.
  * branch0's ragged tail tile is topped up with branch1 channels
    (center-tap only) so the 128-wide PE columns stay productive.
  * per-image work is split in two 28-row slabs, chunked into 7/8-row
    matmul groups (free dim 406/464 <= one PSUM bank).
"""

import numpy as np

import concourse.mybir as mybir
import concourse.tile as tile
from concourse import bacc
from concourse.bass_utils import run_bass_kernel_spmd

# ---- problem constants (hardcoded per spec) ----
B, C, H, W = 16, 512, 56, 56
MID = 256
NCORES = 8
BL = B // NCORES  # images per core
WP = H + 2  # padded row width 58
SLAB_ROWS = 28
BUF = 30 * WP + 2  # slab buffer free size (30 padded rows + 2 slack) = 1742
EPS = 1e-5
CENTER_FIRST = [4, 0, 1, 2, 3, 5, 6, 7, 8]  # tap order: center tap starts PSUM

F32 = mybir.dt.float32
F32R = mybir.dt.float32r
AF = mybir.ActivationFunctionType

PROFILE = False
LAST_EXEC_NS = None
_CACHE = {}


def _tiles_of(M):
    out = []
    o = 0
    while o < M:
        out.append((o, min(128, M - o)))
        o += 128
    return out


def _e_split(M0):
    """b0 tail: use tap-expansion when the ragged remainder is small."""
    rem9 = M0 % 128
    use_e = 0 < rem9 <= 28
    return (M0 - rem9, rem9) if use_e else (M0, 0)


def _par_cols(counts):
    M0, M1, M2, M3 = counts
    n2, n3 = len(_tiles_of(M2)), len(_tiles_of(M3))
    cols = {"s31": 0, "t31": 2}
    c = 4
    cols["a_s1"] = c
    c += n3
    cols["a_t1"] = c
    c += n3
    cols["bias2"] = c
    c += n2
    cols["pscale"] = c
    c += n3
    cols["pbias"] = c
    c += n3
    return cols, c


# ---------------- host-side folding ----------------

def _bn_fold(p):
    g, b, m, v = [p[i].astype(np.float64) for i in range(4)]
    s = g / np.sqrt(v + EPS)
    t = b - m * s
    return s.astype(np.float32), t.astype(np.float32)


def _prep(w_main, w_1x1, w31, bn31, w33, bn33, wa1, bna1, bna2, fuse_weight, c_score):
    f = [float(fuse_weight[i]) for i in range(4)]
    s31, t31 = _bn_fold(bn31)
    s33, t33 = _bn_fold(bn33)
    sa1, ta1 = _bn_fold(bna1)
    sa2, ta2 = _bn_fold(bna2)

    ind = np.argsort(-c_score, kind="stable")[:C]
    scale = (1.0 / (1.0 + np.exp(-c_score.astype(np.float64))))[ind].astype(np.float32)

    js = {b_: [] for b_ in range(4)}
    cs = {b_: [] for b_ in range(4)}
    for j, gi in enumerate(ind):
        b_ = int(gi) // 256
        js[b_].append(j)
        cs[b_].append(int(gi) % 256)
    c0, c1, c2, c3 = [np.array(cs[i], np.int64) for i in range(4)]
    sc = [scale[np.array(js[i], np.int64)] if js[i] else np.zeros(0, np.float32) for i in range(4)]

    W0 = w_main[c0] * f[0] * sc[0][:, None, None, None]            # [M0,C,3,3]
    W1 = w_1x1[c1, :, 0, 0] * f[1] * sc[1][:, None]                # [M1,C]
    W31 = w31[:, :, 0, 0] * f[2]                                   # [MID,C]
    W33 = w33[c2] * (s33[c2] * sc[2])[:, None, None, None]         # [M2,MID,3,3]
    bias2 = t33[c2] * sc[2]
    Wa1 = wa1[c3, :, 0, 0] * f[3]                                  # [M3,C]
    a_s1, a_t1 = sa1[c3], ta1[c3]
    pscale = sa2[c3] * sc[3] / 9.0
    pbias = ta2[c3] * sc[3]

    jperm = np.array(js[0] + js[1] + js[2] + js[3], dtype=np.int64)
    counts = (len(c0), len(c1), len(c2), len(c3))
    M0, M1, M2, M3 = counts

    # branch0: full 128-wide tiles; small ragged tail handled via tap
    # expansion (e-channels) instead of a nearly-empty 9-tap tile
    rem9 = M0 % 128
    use_e = 0 < rem9 <= 28
    M0F = M0 - rem9 if use_e else M0
    W0F = W0[:M0F]
    if use_e:
        W0T = W0[M0F:]  # [rem9, C, 3, 3]
        # e-channel 1x1 weights: channel (t*rem9 + j) = tap t of tail ch j
        Ew = np.ascontiguousarray(
            W0T.transpose(2, 3, 0, 1).reshape(9 * rem9, C)
        )
        # selector for the tail conv: lhsT[i, t, j] = 1 iff i == t*rem9+j
        sel = np.zeros((128, 9, rem9), np.float32)
        for t_ in range(9):
            for j_ in range(rem9):
                sel[t_ * rem9 + j_, t_, j_] = 1.0

    def pack_kpm(Wmat, ktiles):
        M = Wmat.shape[0]
        return np.ascontiguousarray(
            Wmat.T.reshape(ktiles, 128, M).transpose(1, 0, 2).reshape(128, ktiles * M)
        ).astype(np.float32)

    def pack_ktpm(Wconv, ktiles):
        M = Wconv.shape[0]
        return np.ascontiguousarray(
            Wconv.transpose(1, 2, 3, 0)
            .reshape(ktiles, 128, 9, M)
            .transpose(1, 0, 2, 3)
            .reshape(128, ktiles * 9 * M)
        ).astype(np.float32)

    arrs = {"W31d": pack_kpm(W31, 4)}
    if M0F:
        arrs["W0d"] = pack_ktpm(W0F, 4)
    if use_e:
        arrs["Ewd"] = pack_kpm(Ew, 4)
        arrs["seld"] = np.ascontiguousarray(sel.reshape(128, 9 * rem9))
    if M1:
        arrs["W1d"] = pack_kpm(W1, 4)
    if M2:
        arrs["W33d"] = pack_ktpm(W33, 2)
    if M3:
        arrs["Wa1d"] = pack_kpm(Wa1, 4)

    cols, ncol = _par_cols(counts)
    par = np.zeros((128, ncol), np.float32)
    par[:, cols["s31"]], par[:, cols["s31"] + 1] = s31[:128], s31[128:]
    par[:, cols["t31"]], par[:, cols["t31"] + 1] = t31[:128], t31[128:]
    for o, (mo, mw) in enumerate(_tiles_of(M3)):
        par[:mw, cols["a_s1"] + o] = a_s1[mo : mo + mw]
        par[:mw, cols["a_t1"] + o] = a_t1[mo : mo + mw]
        par[:mw, cols["pscale"] + o] = pscale[mo : mo + mw]
        par[:mw, cols["pbias"] + o] = pbias[mo : mo + mw]
    for o, (mo, mw) in enumerate(_tiles_of(M2)):
        par[:mw, cols["bias2"] + o] = bias2[mo : mo + mw]
    arrs["par"] = par
    return arrs, counts, jperm


# ---------------- device graph ----------------

def _build(counts):
    M0, M1, M2, M3 = counts
    cols, ncol = _par_cols(counts)
    M0F, rem9 = _e_split(M0)
    NE = 9 * rem9  # e-channel count
    n3tiles = _tiles_of(M3)
    nc = bacc.Bacc("TRN2", target_bir_lowering=False, debug=False, num_devices=NCORES)

    x_ext = nc.dram_tensor("x", [BL, C, H, W], F32R, kind="ExternalInput")
    W0_ext = nc.dram_tensor("W0d", [128, 4 * 9 * M0F], F32R, kind="ExternalInput") if M0F else None
    Ew_ext = nc.dram_tensor("Ewd", [128, 4 * NE], F32R, kind="ExternalInput") if rem9 else None
    sel_ext = nc.dram_tensor("seld", [128, NE], F32R, kind="ExternalInput") if rem9 else None
    W1_ext = nc.dram_tensor("W1d", [128, 4 * M1], F32R, kind="ExternalInput") if M1 else None
    W31_ext = nc.dram_tensor("W31d", [128, 4 * MID], F32R, kind="ExternalInput")
    W33_ext = nc.dram_tensor("W33d", [128, 2 * 9 * M2], F32R, kind="ExternalInput") if M2 else None
    Wa1_ext = nc.dram_tensor("Wa1d", [128, 4 * M3], F32R, kind="ExternalInput") if M3 else None
    par_ext = nc.dram_tensor("par", [128, ncol], F32, kind="ExternalInput")
    out_ext = nc.dram_tensor("out", [BL, C, H, W], F32, kind="ExternalOutput")

    off1, off2, off3 = M0, M0 + M1, M0 + M1 + M2

    with tile.TileContext(nc) as tc:
        with (
            tc.tile_pool(name="wpool", bufs=1) as wpool,
            tc.tile_pool(name="acts", bufs=1) as acts,
            tc.tile_pool(name="stage", bufs=4) as stage,
            tc.tile_pool(name="xstage", bufs=4) as xstage,
            tc.tile_pool(name="slabstage", bufs=1) as slabstage,
            tc.tile_pool(name="ps", bufs=8, space="PSUM") as ps,
        ):
            # ---- persistent weights (order: first consumers first) ----
            part = wpool.tile([128, ncol], F32)
            wt31 = wpool.tile([128, 4, 2, 128], F32R)
            wta1 = wpool.tile([128, 4, M3], F32R, name="wta1") if M3 else None
            wt0 = wpool.tile([128, 4, 9, M0F], F32R, name="wt0") if M0F else None
            wte = wpool.tile([128, 4, NE], F32R, name="wte") if rem9 else None
            selt = wpool.tile([128, 9, rem9], F32R, name="selt") if rem9 else None
            wt1 = wpool.tile([128, 4, M1], F32R, name="wt1") if M1 else None
            wt33 = wpool.tile([128, 2, 9, M2], F32R, name="wt33") if M2 else None
            nc.sync.dma_start(part[:], par_ext.ap())
            nc.sync.dma_start(wt31[:], W31_ext.ap().rearrange("p (k o m) -> p k o m", k=4, o=2))
            if M3:
                nc.gpsimd.dma_start(wta1[:], Wa1_ext.ap().rearrange("p (k m) -> p k m", k=4))
            if M0F:
                nc.scalar.dma_start(wt0[:], W0_ext.ap().rearrange("p (k t m) -> p k t m", k=4, t=9))
            if rem9:
                nc.gpsimd.dma_start(wte[:], Ew_ext.ap().rearrange("p (k m) -> p k m", k=4))
                nc.gpsimd.dma_start(selt[:], sel_ext.ap().rearrange("p (t m) -> p t m", t=9))
            if M1:
                nc.gpsimd.dma_start(wt1[:], W1_ext.ap().rearrange("p (k m) -> p k m", k=4))
            if M2:
                nc.scalar.dma_start(wt33[:], W33_ext.ap().rearrange("p (k t m) -> p k t m", k=2, t=9))

            # ---- persistent activation buffers (2 slab slots each) ----
            xt = [[acts.tile([128, BUF], F32R, name=f"xt{k}{s}", tag=f"x{k}s{s}") for s in range(2)] for k in range(4)]
            z1 = [[acts.tile([128, BUF], F32R, name=f"z1{k}{s}", tag=f"z{k}s{s}") for s in range(2)] for k in range(2)]
            za = [[acts.tile([128, BUF], F32, name=f"za{o}{s}", tag=f"za{o}s{s}") for s in range(2)] for o in range(len(n3tiles))]
            ze = [acts.tile([128, BUF], F32R, name=f"ze{s}", tag=f"zes{s}") for s in range(2)] if rem9 else None
            hs = [acts.tile([128, BUF], F32, name="hs0", tag="hs0")] * max(1, len(n3tiles))

            # zero only the pad regions (row pads, col pads, slack), not the
            # whole buffers: three tiny memsets per buffer, split over engines.
            def pad_memsets(t, eng):
                a = t[:].bitcast(mybir.dt.uint32)
                eng.memset(a[:, 0:59], 0)  # slack + row 0
                # col pads: w in {0,57} of every row == flat {58r, 58r+1}
                eng.memset(a[:, 0 : 30 * WP].rearrange("p (r w) -> p r w", w=WP)[:, :, 0:2], 0)
                eng.memset(a[:, 29 * WP + 1 : BUF], 0)  # row 29 + tail slack

            for sidx in range(2):  # slot-0 buffers gate the first slab; do them first
                for group in (xt, z1, za, [ze] if rem9 else []):
                    for pair in group:
                        pad_memsets(pair[sidx], nc.vector)

            OUT_STARTS = [1, 8, 15, 22]  # slab-local output row starts (7 rows)

            def evict_to_out(acc, mw, segs, b, g0):
                """PSUM rows [0,mw) -> valid cols -> stage; then one DMA per
                (p_lo, p_hi, ch0) segment (PSUM reads must start at part 0)."""
                st = stage.tile([mw, 7 * 56], F32, name="st", tag="st")
                src = acc[0:mw].rearrange("p (r w) -> p r w", w=WP)[:, :, 1:57]
                dst = st[:].rearrange("p (r w) -> p r w", w=56)
                nc.scalar.activation(dst, src, AF.Copy)
                for (p_lo, p_hi, ch0) in segs:
                    nc.sync.dma_start(
                        out_ext.ap()[b, ch0 : ch0 + p_hi - p_lo, g0 : g0 + 7, :],
                        st[p_lo:p_hi].rearrange("p (r w) -> p r w", w=56),
                    )

            for b in range(BL):
                for s in range(2):
                    # ---- DMA x slab into padded layout (split in 2 pieces) ----
                    src_r0 = 0 if s == 0 else 27
                    l0 = 1 - s
                    x_pieces = [(1, 7), (8, 7), (15, 7), (22, 8)] if s == 0 else [(0, 7), (7, 7), (14, 7), (21, 8)]
                    for (lp, pn) in x_pieces:
                        for k in range(4):
                            xs = xstage.tile([128, 8 * 56], F32R, name="xs", tag="xs")
                            nc.sync.dma_start(
                                xs[:, 0 : pn * 56],
                                x_ext.ap()[b, 128 * k : 128 * (k + 1),
                                           SLAB_ROWS * s + lp - 1 : SLAB_ROWS * s + lp - 1 + pn, :],
                            )
                            dst = xt[k][s][:, lp * WP + 2 : lp * WP + 2 + pn * WP].rearrange(
                                "p (r w) -> p r w", w=WP
                            )[:, :, 0:56]
                            srcv = xs[:, 0 : pn * 56].rearrange("p (r w) -> p r w", w=56)
                            if k % 2 == 0:
                                nc.vector.tensor_copy(dst, srcv)
                            else:
                                nc.scalar.activation(dst, srcv, AF.Copy)

                    if s == 0:
                        mid_chunks = [(1, 7), (8, 7), (15, 7), (22, 8)]
                    else:
                        mid_chunks = [(0, 7), (7, 7), (14, 7), (21, 8)]

                    # ---- branch2 conv1: x -> z1 (full MID channels) ----
                    for (l0c, nr) in mid_chunks:
                        q0 = l0c * WP + 1
                        N = nr * WP
                        for o in range(2):
                            acc = ps.tile([128, N], F32, name="acc", tag="ps")
                            for k in range(4):
                                nc.tensor.matmul(
                                    acc[:], wt31[:, k, o, :], xt[k][s][:, q0 : q0 + N],
                                    start=(k == 0), stop=(k == 3),
                                )
                            src = acc[:].rearrange("p (r w) -> p r w", w=WP)[:, :, 1:57]
                            dst = z1[o][s][:, l0c * WP + 2 : l0c * WP + 2 + N].rearrange(
                                "p (r w) -> p r w", w=WP
                            )[:, :, 0:56]
                            nc.scalar.activation(
                                dst, src, AF.Identity,
                                bias=part[:, cols["t31"] + o : cols["t31"] + o + 1],
                                scale=part[:, cols["s31"] + o : cols["s31"] + o + 1],
                            )

                    # ---- branch3 conv1: x -> za ----
                    for o, (mo, mw) in enumerate(n3tiles):
                        for (l0c, nr) in mid_chunks:
                            q0 = l0c * WP + 1
                            N = nr * WP
                            acc = ps.tile([mw, N], F32, name="acc", tag="ps")
                            for k in range(4):
                                nc.tensor.matmul(
                                    acc[:], wta1[:, k, mo : mo + mw], xt[k][s][:, q0 : q0 + N],
                                    start=(k == 0), stop=(k == 3),
                                )
                            src = acc[:].rearrange("p (r w) -> p r w", w=WP)[:, :, 1:57]
                            dst = za[o][s][:mw, l0c * WP + 2 : l0c * WP + 2 + N].rearrange(
                                "p (r w) -> p r w", w=WP
                            )[:, :, 0:56]
                            nc.scalar.activation(
                                dst, src, AF.Identity,
                                bias=part[:mw, cols["a_t1"] + o : cols["a_t1"] + o + 1],
                                scale=part[:mw, cols["a_s1"] + o : cols["a_s1"] + o + 1],
                            )

                    # ---- e-channels for b0's tail (1x1 conv on mid-chunks) ----
                    if rem9:
                        for (l0c, nr) in mid_chunks:
                            q0 = l0c * WP + 1
                            N = nr * WP
                            acc = ps.tile([NE, N], F32, name="acc", tag="ps")
                            for k in range(4):
                                nc.tensor.matmul(
                                    acc[:], wte[:, k, :], xt[k][s][:, q0 : q0 + N],
                                    start=(k == 0), stop=(k == 3),
                                )
                            src = acc[:].rearrange("p (r w) -> p r w", w=WP)[:, :, 1:57]
                            dst = ze[s][:NE, l0c * WP + 2 : l0c * WP + 2 + N].rearrange(
                                "p (r w) -> p r w", w=WP
                            )[:, :, 0:56]
                            nc.scalar.activation(dst, src, AF.Copy)

                    # ---- branch3 pooling (VectorE separable 3x3 sum) ----
                    add = mybir.AluOpType.add
                    for o, (mo, mw) in enumerate(n3tiles):
                        zz = za[o][s]
                        hh = hs[o]
                        nc.vector.tensor_tensor(
                            hh[:mw, 1 : BUF - 1], zz[:mw, 0 : BUF - 2], zz[:mw, 2:BUF], add
                        )
                        nc.vector.tensor_tensor(
                            hh[:mw, 1 : BUF - 1], hh[:mw, 1 : BUF - 1], zz[:mw, 1 : BUF - 1], add
                        )

                        def v3(buf, base, mwl=mw):
                            return buf[:mwl, base : base + 28 * WP].rearrange(
                                "p (r w) -> p r w", w=WP
                            )[:, :, 0:56]

                        # vsum over valid cols only, in place over za
                        nc.vector.tensor_tensor(v3(zz, 60), v3(hh, 2), v3(hh, 118), add)
                        nc.vector.tensor_tensor(v3(zz, 60), v3(zz, 60), v3(hh, 60), add)
                        st = slabstage.tile([mw, 28 * 56], F32, name="sst", tag="sst")
                        nc.scalar.activation(
                            st[:].rearrange("p (r w) -> p r w", w=56),
                            v3(zz, 60), AF.Identity,
                            bias=part[:mw, cols["pbias"] + o : cols["pbias"] + o + 1],
                            scale=part[:mw, cols["pscale"] + o : cols["pscale"] + o + 1],
                        )
                        g0 = SLAB_ROWS * s
                        nc.sync.dma_start(
                            out_ext.ap()[b, off3 + mo : off3 + mo + mw, g0 : g0 + 28, :],
                            st[:].rearrange("p (r w) -> p r w", w=56),
                        )

                    # ---- branch0 (+b1 tail riders) and branch1-rest (1x1) ----
                    for l0c in OUT_STARTS:
                        q0 = l0c * WP + 1
                        N = 7 * WP
                        g0 = l0c - 1 + SLAB_ROWS * s
                        for (mo, mw) in _tiles_of(M0F):
                            acc = ps.tile([mw, N], F32, name="acc", tag="ps")
                            first = True
                            for t in CENTER_FIRST:
                                dh, dw = t // 3, t % 3
                                qr = q0 + (dh - 1) * WP + (dw - 1)
                                for k in range(4):
                                    nc.tensor.matmul(
                                        acc[:], wt0[:, k, t, mo : mo + mw],
                                        xt[k][s][:, qr : qr + N],
                                        start=first, stop=(t == 8 and k == 3),
                                    )
                                    first = False
                            evict_to_out(acc, mw, [(0, mw, mo)], b, g0)
                        if rem9:
                            # tail channels: 9-tap conv over the e-buffer with
                            # one-hot selector weights (K = NE)
                            acc = ps.tile([rem9, N], F32, name="acc", tag="ps")
                            first = True
                            for t in CENTER_FIRST:
                                dh, dw = t // 3, t % 3
                                qr = q0 + (dh - 1) * WP + (dw - 1)
                                nc.tensor.matmul(
                                    acc[:], selt[:NE, t, :], ze[s][:NE, qr : qr + N],
                                    start=first, stop=(t == 8),
                                )
                                first = False
                            evict_to_out(acc, rem9, [(0, rem9, M0F)], b, g0)
                        for (mo, mw) in _tiles_of(M1):
                            acc = ps.tile([mw, N], F32, name="acc", tag="ps")
                            for k in range(4):
                                nc.tensor.matmul(
                                    acc[:], wt1[:, k, mo : mo + mw], xt[k][s][:, q0 : q0 + N],
                                    start=(k == 0), stop=(k == 3),
                                )
                            evict_to_out(acc, mw, [(0, mw, off1 + mo)], b, g0)

                    # ---- branch2 conv2 (3x3 on z1) ----
                    for l0c in OUT_STARTS:
                        q0 = l0c * WP + 1
                        N = 7 * WP
                        g0 = l0c - 1 + SLAB_ROWS * s
                        for o, (mo, mw) in enumerate(_tiles_of(M2)):
                            acc = ps.tile([mw, N], F32, name="acc", tag="ps")
                            first = True
                            for t in CENTER_FIRST:
                                dh, dw = t // 3, t % 3
                                qr = q0 + (dh - 1) * WP + (dw - 1)
                                for k in range(2):
                                    nc.tensor.matmul(
                                        acc[:], wt33[:, k, t, mo : mo + mw],
                                        z1[k][s][:, qr : qr + N],
                                        start=first, stop=(t == 8 and k == 1),
                                    )
                                    first = False
                            st = stage.tile([mw, 7 * 56], F32, name="st", tag="st")
                            nc.scalar.activation(
                                st[:].rearrange("p (r w) -> p r w", w=56),
                                acc[:].rearrange("p (r w) -> p r w", w=WP)[:, :, 1:57],
                                AF.Identity,
                                bias=part[:mw, cols["bias2"] + o : cols["bias2"] + o + 1],
                            )
                            nc.sync.dma_start(
                                out_ext.ap()[b, off2 + mo : off2 + mo + mw, g0 : g0 + 7, :],
                                st[:].rearrange("p (r w) -> p r w", w=56),
                            )

    nc.compile()
    return nc


# ---------------- entry point ----------------

def kernel(x, w_main, w_1x1, w31, bn31, w33, bn33, wa1, bna1, bna2, fuse_weight, c_score):
    global LAST_EXEC_NS
    arrs, counts, jperm = _prep(
        w_main, w_1x1, w31, bn31, w33, bn33, wa1, bna1, bna2, fuse_weight, c_score
    )
    if counts not in _CACHE:
        _CACHE[counts] = _build(counts)
    nc = _CACHE[counts]

    x = np.ascontiguousarray(x, dtype=np.float32)
    in_maps = []
    for i in range(NCORES):
        m = {"x": np.ascontiguousarray(x[BL * i : BL * (i + 1)])}
        m.update(arrs)
        in_maps.append(m)

    res = run_bass_kernel_spmd(nc, in_maps, list(range(NCORES)), trace=PROFILE)
    LAST_EXEC_NS = res.exec_time_ns

    full = np.empty((B, C, H, W), np.float32)
    for i in range(NCORES):
        full[BL * i : BL * (i + 1)] = res.results[i]["out"]
    out = np.empty_like(full)
    out[:, jperm] = full
    return out
